# revision 8
# baseline (speedup 1.0000x reference)
"""Trainium2 Bass kernel for a 2-layer tanh RNN + final FC.

Reference computation (PyTorch-style RNN, batch_first):
  layer l: h_t = tanh(x_t @ W_ih^T + b_ih + b_hh + h_{t-1} @ W_hh^T)
  out = h1_T @ W_fc^T + b_fc          (final hidden of layer 1)

Shapes: x [64, 1024, 256], H=512, O=256.

Sharding: data-parallel over batch across 8 cores (8 samples/core);
weights replicated (all fp16 on device, fp32 PSUM accumulation).

Per-core design — everything stays in "hidden-on-partition" layout so
the sequential recurrence never needs a transpose:

* State h_l[t] lives as fp16 SBUF tiles [128, KH, BC] (partition p of
  k-slice k is hidden unit 128k+p).

* Per 16-step window and layer, the input projection
  xw = W_ih^T.T @ x (+bias via a K=1 ones-matmul) is computed directly
  INTO a PSUM bank [128, KH, S, BC].  The per-step recurrence matmuls
  z += W_hh tile.T @ h_{t-1} then accumulate on top (start=False), so
  a single ScalarE tanh per step reads PSUM and writes h_t — the
  critical path is just PE -> ACT -> PE.

* The stationary operands are full [128,128] fp16 weight tiles, which
  triggers the compiler's fast-weight-load path; the moving operands
  are the tiny [128, BC] h slices.

* Layer 1 is pipelined one window (16 steps) behind layer 0; the two
  recurrence chains interleave on PE/ACT so sync latency is hidden.
"""

import sys

if "/opt/trn_rl_repo" not in sys.path:
    sys.path.insert(0, "/opt/trn_rl_repo")

import numpy as np

import concourse.bacc as bacc
import concourse.mybir as mybir
import concourse.tile as tile
from concourse import bass_utils

F16 = mybir.dt.float16
F32 = mybir.dt.float32
AF = mybir.ActivationFunctionType

N_CORES = 8
B, T, D, H, O = 64, 1024, 256, 512, 256
BC = B // N_CORES  # batch per core
S = 16  # timesteps per window (one PSUM bank per layer)
KH = H // 128  # 4
KD = D // 128  # 2
MO = O // 128  # 2


def build(T=T, reps=1):
    """Build the per-core Bass program. reps>1 re-runs the whole body
    (timing amplification only)."""
    NW = T // S  # number of windows
    NB = S * BC  # columns per window in (t, b) order (128)

    nc = bacc.Bacc("TRN2", target_bir_lowering=False, debug=False,
                   num_devices=N_CORES)

    xT_d = nc.dram_tensor("xT", [D, T * BC], F16, kind="ExternalInput")
    wih0_d = nc.dram_tensor("wih0T", [D, H], F16, kind="ExternalInput")
    whh0_d = nc.dram_tensor("whh0T", [H, H], F16, kind="ExternalInput")
    wih1_d = nc.dram_tensor("wih1T", [H, H], F16, kind="ExternalInput")
    whh1_d = nc.dram_tensor("whh1T", [H, H], F16, kind="ExternalInput")
    wfc_d = nc.dram_tensor("wfcT", [H, O], F16, kind="ExternalInput")
    b0_d = nc.dram_tensor("b0", [1, H], F16, kind="ExternalInput")
    b1_d = nc.dram_tensor("b1", [1, H], F16, kind="ExternalInput")
    bfc_d = nc.dram_tensor("bfc", [128, MO], F32, kind="ExternalInput")
    out_d = nc.dram_tensor("out", [O, BC], F32, kind="ExternalOutput")

    with tile.TileContext(nc) as tc:
        with (
            tc.tile_pool(name="wpool", bufs=1) as wpool,
            tc.tile_pool(name="xpool", bufs=3) as xpool,
            tc.tile_pool(name="h0pool", bufs=2) as h0pool,
            tc.tile_pool(name="h1pool", bufs=2) as h1pool,
            tc.tile_pool(name="spool", bufs=2) as spool,
            tc.tile_pool(name="ps0", bufs=2, space="PSUM") as ps0pool,
            tc.tile_pool(name="ps1", bufs=2, space="PSUM") as ps1pool,
            tc.tile_pool(name="psf", bufs=1, space="PSUM") as psfpool,
        ):
            # ---- weight preload (SBUF-resident, fp16) ----
            # layout: W^T as [128, kt*mt*128]; (k, m) stationary subtile at
            # columns (k*mt + m)*128.
            def load_w(dram, kt, mt, name):
                wt = wpool.tile([128, kt * mt * 128], F16, name=name)
                for k in range(kt):
                    nc.sync.dma_start(
                        wt[:, k * mt * 128:(k + 1) * mt * 128],
                        dram[k * 128:(k + 1) * 128, :],
                    )
                return wt

            def wsl(wt, k, m, mt):
                c = (k * mt + m) * 128
                return wt[:, c:c + 128]

            wih0 = load_w(wih0_d, KD, KH, "wih0")
            whh0 = load_w(whh0_d, KH, KH, "whh0")
            wih1 = load_w(wih1_d, KH, KH, "wih1")
            whh1 = load_w(whh1_d, KH, KH, "whh1")
            wfc = load_w(wfc_d, KH, MO, "wfc")
            b0 = wpool.tile([1, H], F16, name="b0s")
            nc.sync.dma_start(b0[:], b0_d[:])
            b1 = wpool.tile([1, H], F16, name="b1s")
            nc.sync.dma_start(b1[:], b1_d[:])
            bfc = wpool.tile([128, MO], F32, name="bfcs")
            nc.sync.dma_start(bfc[:], bfc_d[:])
            ones = wpool.tile([1, NB], F16, name="ones")
            nc.vector.memset(ones[:], 1.0)
            hz = wpool.tile([128, KH, BC], F16, name="hzero")
            nc.vector.memset(hz[:], 0.0)

            def dma_x(w):
                xc = xpool.tile([128, KD, NB], F16, name="xc")
                for k in range(KD):
                    nc.gpsimd.dma_start(
                        xc[:, k],
                        xT_d[k * 128:(k + 1) * 128, w * NB:(w + 1) * NB],
                    )
                return xc

            # PSUM zero-region discipline: each window tile is exactly one
            # 2KB bank; the FIRST matmul into it (and only that one) uses
            # start=True — it marks the whole bank pending-zero, so every
            # later matmul's first touch of a byte initializes it and
            # subsequent touches accumulate.  The very last matmul of the
            # window (t=S-1, m=KH-1, k=KH-1) closes the group (stop=True).
            def proj(ps, wt, kt, bias, rhs_slices):
                """ps[:, m, t, b] = bias[m*128+p] + sum_k wt[k,m].T @ rhs"""
                for m in range(KH):
                    nc.tensor.matmul(
                        ps[:, m], bias[0:1, m * 128:(m + 1) * 128], ones[:],
                        start=(m == 0), stop=False,
                    )
                    for k in range(kt):
                        nc.tensor.matmul(
                            ps[:, m], wsl(wt, k, m, KH), rhs_slices(k),
                            start=False, stop=False,
                        )

            def step(ps, whh, hprev, hcur, t):
                """hcur[:, t] = tanh(ps[:, :, t] + W_hh @ hprev)

                The t=0 last matmul carries a real stop=True: it closes the
                bank's group (clearing started_view) so tanhs may read
                PSUM.  All other step matmuls skip the group bookkeeping;
                values still accumulate correctly because the bank's
                pending-zero bytes were already consumed by the proj."""
                for m in range(KH):
                    for k in range(KH):
                        closer = t == 0 and m == KH - 1 and k == KH - 1
                        nc.tensor.matmul(
                            ps[:, m, t], wsl(whh, k, m, KH), hprev[:, k],
                            start=False,
                            stop=closer,
                            skip_group_check=not closer,
                        )
                nc.scalar.activation(hcur[:, t], ps[:, :, t, :], AF.Tanh)

            for _rep in range(reps):
                xc_tiles = {}
                ps0_tiles = {}
                hc0_prev = hc1_prev = hc1 = None
                for w in range(2):
                    if w < NW:
                        xc_tiles[w] = dma_x(w)
                ps0_tiles[0] = ps0pool.tile([128, KH, S, BC], F32, name="ps0t")
                proj(ps0_tiles[0], wih0, KD, b0,
                     lambda k: xc_tiles[0][:, k])

                for wi in range(NW + 1):
                    l0 = wi < NW
                    l1 = wi >= 1
                    if wi + 2 < NW:
                        xc_tiles[wi + 2] = dma_x(wi + 2)
                    if wi + 1 < NW:
                        ps0_tiles[wi + 1] = ps0pool.tile(
                            [128, KH, S, BC], F32, name="ps0t")
                        xcn = xc_tiles[wi + 1]
                        proj(ps0_tiles[wi + 1], wih0, KD, b0,
                             lambda k: xcn[:, k])
                        xc_tiles.pop(wi, None)
                    if l1:
                        ps1 = ps1pool.tile([128, KH, S, BC], F32, name="ps1t")
                        h0w = hc0_prev
                        proj(ps1, wih1, KH, b1,
                             lambda k: h0w[:, :, k, :])
                        hc1_prev = hc1
                        hc1 = h1pool.tile([128, S, KH, BC], F16, name="hc1")
                    if l0:
                        ps0 = ps0_tiles.pop(wi)
                        hc0_new = h0pool.tile([128, S, KH, BC], F16,
                                              name="hc0")
                    for t in range(S):
                        if l1:
                            if t == 0:
                                h1p = hz if wi == 1 else hc1_prev[:, S - 1]
                            else:
                                h1p = hc1[:, t - 1]
                            step(ps1, whh1, h1p, hc1, t)
                        if l0:
                            if t == 0:
                                h0p = hz if wi == 0 else hc0_prev[:, S - 1]
                            else:
                                h0p = hc0_new[:, t - 1]
                            step(ps0, whh0, h0p, hc0_new, t)
                    if l0:
                        hc0_prev = hc0_new
                # NOTE: after the loop hc1 holds layer-1 window NW-1.

                # ---- final FC: out^T[o, b] = W_fc[o, :] @ h1_last + b_fc ----
                h1f = hc1[:, S - 1]
                psf = psfpool.tile([128, MO * BC], F32, name="psft")
                for m in range(MO):
                    for k in range(KH):
                        nc.tensor.matmul(
                            psf[:, m * BC:(m + 1) * BC],
                            wsl(wfc, k, m, MO),
                            h1f[:, k, :],
                            start=(m == 0 and k == 0),
                            stop=(m == MO - 1 and k == KH - 1),
                        )
                outs = spool.tile([128, MO * BC], F32, name="outs")
                for m in range(MO):
                    nc.scalar.activation(
                        outs[:, m * BC:(m + 1) * BC],
                        psf[:, m * BC:(m + 1) * BC],
                        AF.Identity,
                        bias=bfc[:, m:m + 1],
                    )
                for m in range(MO):
                    nc.sync.dma_start(out_d[m * 128:(m + 1) * 128, :],
                                      outs[:, m * BC:(m + 1) * BC])

    nc.compile()
    return nc


def make_in_maps(inputs, T=T):
    """Host-side sharding: full inputs -> per-core input dicts."""
    x = np.asarray(inputs["x"], np.float32)

    def t16(a):
        return np.ascontiguousarray(np.asarray(a, np.float32).T
                                    .astype(np.float16))

    shared = {
        "wih0T": t16(inputs["W_ih0"]),
        "whh0T": t16(inputs["W_hh0"]),
        "wih1T": t16(inputs["W_ih1"]),
        "whh1T": t16(inputs["W_hh1"]),
        "wfcT": t16(inputs["W_fc"]),
        "b0": (np.asarray(inputs["b_ih0"], np.float32)
               + np.asarray(inputs["b_hh0"], np.float32))
        .astype(np.float16).reshape(1, H),
        "b1": (np.asarray(inputs["b_ih1"], np.float32)
               + np.asarray(inputs["b_hh1"], np.float32))
        .astype(np.float16).reshape(1, H),
        "bfc": np.ascontiguousarray(
            np.asarray(inputs["b_fc"], np.float32).reshape(MO, 128).T),
    }
    in_maps = []
    for i in range(N_CORES):
        xc = x[i * BC:(i + 1) * BC, :T]  # [BC, T, D]
        xT = np.ascontiguousarray(
            xc.transpose(2, 1, 0).reshape(D, T * BC).astype(np.float16))
        in_maps.append({"xT": xT, **shared})
    return in_maps


def assemble_out(results):
    out = np.empty((B, O), np.float32)
    for i in range(N_CORES):
        out[i * BC:(i + 1) * BC] = results[i]["out"].T
    return out


_NC_CACHE = {}


def kernel(**inputs) -> np.ndarray:
    if "nc" not in _NC_CACHE:
        _NC_CACHE["nc"] = build()
    nc = _NC_CACHE["nc"]
    in_maps = make_in_maps(inputs)
    res = bass_utils.run_bass_kernel_spmd(nc, in_maps, list(range(N_CORES)))
    return assemble_out(res.results)


# revision 17
# speedup vs baseline: 1200.4649x; 1200.4649x over previous
"""Trainium2 Bass kernel for a 2-layer tanh RNN + final FC.

Reference computation (PyTorch-style RNN, batch_first):
  layer l: h_t = tanh(x_t @ W_ih^T + b_ih + b_hh + h_{t-1} @ W_hh^T)
  out = h1_T @ W_fc^T + b_fc          (final hidden of layer 1)

Shapes: x [64, 1024, 256], H=512, O=256.

Sharding: data-parallel over batch across 8 cores (8 samples/core);
weights replicated (fp16 on device, fp32 PSUM accumulation).

Per-core design — everything stays in "hidden-on-partition" layout so
the sequential recurrence never needs a transpose:

* State h_l[t] lives as fp16 SBUF tiles [128, S, KH, BC] (partition p
  of k-slice k is hidden unit 128k+p).

* Per 16-step window and layer, the input projection
  xw = W_ih^T.T @ x (+bias via a K=1 ones-matmul) is computed directly
  INTO a PSUM bank [128, KH, S, BC].  The per-step recurrence matmuls
  z += W_hh tile.T @ h_{t-1} accumulate on top (start=False), so a
  single ScalarE tanh per step per layer reads PSUM and writes h_t —
  the critical path is just PE -> ACT -> PE.

* Stationary operands are full [128,128] fp16 weight tiles (fast
  weight load); moving operands are the tiny [128, BC] h slices.

* The two layers run as independent chains that interleave on PE/ACT,
  hiding the per-step semaphore + pipeline latency.  Layer 1 lags
  layer 0 by TWO windows so both layers' projection matmuls can be
  spread across the step loop (2 per step) instead of bursting at
  window boundaries and head-of-line blocking the in-order PE queue.
"""

import sys

if "/opt/trn_rl_repo" not in sys.path:
    sys.path.insert(0, "/opt/trn_rl_repo")

import numpy as np

import concourse.bacc as bacc
import concourse.mybir as mybir
import concourse.tile as tile
from concourse import bass_utils

F16 = mybir.dt.float16
F32 = mybir.dt.float32
AF = mybir.ActivationFunctionType

N_CORES = 8
B, T, D, H, O = 64, 1024, 256, 512, 256
BC = B // N_CORES
S = 16
KH = H // 128  # 4
KD = D // 128  # 2
MO = O // 128  # 2


def build(T=T, reps=1):
    NW = T // S
    NB = S * BC  # 128

    nc = bacc.Bacc("TRN2", target_bir_lowering=False, debug=False,
                   num_devices=N_CORES)

    xT_d = nc.dram_tensor("xT", [D, T * BC], F16, kind="ExternalInput")
    wih0_d = nc.dram_tensor("wih0T", [D, H], F16, kind="ExternalInput")
    whh0_d = nc.dram_tensor("whh0T", [H, H], F16, kind="ExternalInput")
    wih1_d = nc.dram_tensor("wih1T", [H, H], F16, kind="ExternalInput")
    whh1_d = nc.dram_tensor("whh1T", [H, H], F16, kind="ExternalInput")
    wfc_d = nc.dram_tensor("wfcT", [H, O], F16, kind="ExternalInput")
    b0_d = nc.dram_tensor("b0", [1, H], F16, kind="ExternalInput")
    b1_d = nc.dram_tensor("b1", [1, H], F16, kind="ExternalInput")
    bfc_d = nc.dram_tensor("bfc", [128, MO], F32, kind="ExternalInput")
    out_d = nc.dram_tensor("out", [O, BC], F32, kind="ExternalOutput")

    with tile.TileContext(nc) as tc:
        with (
            tc.tile_pool(name="wpool", bufs=1) as wpool,
            tc.tile_pool(name="xpool", bufs=4) as xpool,
            tc.tile_pool(name="h0pool", bufs=2) as h0pool,
            tc.tile_pool(name="h1pool", bufs=2) as h1pool,
            tc.tile_pool(name="spool", bufs=2) as spool,
            tc.tile_pool(name="ps0", bufs=2, space="PSUM") as ps0pool,
            tc.tile_pool(name="ps1", bufs=2, space="PSUM") as ps1pool,
            tc.tile_pool(name="psf", bufs=1, space="PSUM") as psfpool,
        ):
            def load_w(dram, kt, mt, name):
                wt = wpool.tile([128, kt * mt * 128], F16, name=name)
                for k in range(kt):
                    nc.sync.dma_start(
                        wt[:, k * mt * 128:(k + 1) * mt * 128],
                        dram[k * 128:(k + 1) * 128, :],
                    )
                return wt

            def wsl(wt, k, m, mt):
                c = (k * mt + m) * 128
                return wt[:, c:c + 128]

            wih0 = load_w(wih0_d, KD, KH, "wih0")
            whh0 = load_w(whh0_d, KH, KH, "whh0")
            wih1 = load_w(wih1_d, KH, KH, "wih1")
            whh1 = load_w(whh1_d, KH, KH, "whh1")
            wfc = load_w(wfc_d, KH, MO, "wfc")
            b0 = wpool.tile([1, H], F16, name="b0s")
            nc.sync.dma_start(b0[:], b0_d[:])
            b1 = wpool.tile([1, H], F16, name="b1s")
            nc.sync.dma_start(b1[:], b1_d[:])
            bfc = wpool.tile([128, MO], F32, name="bfcs")
            nc.sync.dma_start(bfc[:], bfc_d[:])
            ones = wpool.tile([1, NB], F16, name="ones")
            nc.vector.memset(ones[:], 1.0)
            hz = wpool.tile([128, KH, BC], F16, name="hzero")
            nc.vector.memset(hz[:], 0.0)

            def dma_x(w):
                xc = xpool.tile([128, KD, NB], F16, name="xc")
                for k in range(KD):
                    nc.gpsimd.dma_start(
                        xc[:, k],
                        xT_d[k * 128:(k + 1) * 128, w * NB:(w + 1) * NB],
                    )
                return xc

            def proj_thunks(ps, wt, kt, bias, rhs_slices):
                """Deferred emission of one window's projection matmuls.
                The first (bias m=0) matmul opens the bank's zero region."""
                ths = []
                for m in range(KH):
                    def bias_mm(m=m):
                        nc.tensor.matmul(
                            ps[:, m], bias[0:1, m * 128:(m + 1) * 128],
                            ones[:], start=(m == 0), stop=False,
                        )
                    ths.append(bias_mm)
                    for k in range(kt):
                        def w_mm(m=m, k=k):
                            nc.tensor.matmul(
                                ps[:, m], wsl(wt, k, m, KH), rhs_slices(k),
                                start=False, stop=False,
                            )
                        ths.append(w_mm)
                return ths

            def step(ps, whh, hprev, hcur, t):
                for m in range(KH):
                    for k in range(KH):
                        closer = t == 0 and m == KH - 1 and k == KH - 1
                        nc.tensor.matmul(
                            ps[:, m, t], wsl(whh, k, m, KH), hprev[:, k],
                            start=False,
                            stop=closer,
                            skip_group_check=not closer,
                        )
                nc.scalar.activation(hcur[:, t], ps[:, :, t, :], AF.Tanh)

            for _rep in range(reps):
                xc_tiles = {}
                ps0_tiles = {}
                ps1_tiles = {}
                hc0_tiles = {}
                hc1_prev = hc1 = None
                for w in range(2):
                    if w < NW:
                        xc_tiles[w] = dma_x(w)
                ps0_tiles[0] = ps0pool.tile([128, KH, S, BC], F32,
                                            name="ps0t")
                for th in proj_thunks(ps0_tiles[0], wih0, KD, b0,
                                      lambda k, xc=xc_tiles[0]: xc[:, k]):
                    th()

                # L1 lags TWO windows: at iter wi, L0 runs window wi and
                # L1 runs window wi-2; proj0(wi+1) and proj1(wi-1) are
                # spread across the step loop.
                for wi in range(NW + 2):
                    l0 = wi < NW
                    l1 = 2 <= wi
                    if wi + 2 < NW:
                        xc_tiles[wi + 2] = dma_x(wi + 2)
                    thunks = []
                    if wi + 1 < NW:
                        ps0_tiles[wi + 1] = ps0pool.tile(
                            [128, KH, S, BC], F32, name="ps0t")
                        xcn = xc_tiles.pop(wi + 1)
                        thunks += proj_thunks(
                            ps0_tiles[wi + 1], wih0, KD, b0,
                            lambda k, xc=xcn: xc[:, k])
                    if 1 <= wi <= NW:
                        ps1_tiles[wi - 1] = ps1pool.tile(
                            [128, KH, S, BC], F32, name="ps1t")
                        h0w = hc0_tiles[wi - 1]
                        thunks += proj_thunks(
                            ps1_tiles[wi - 1], wih1, KH, b1,
                            lambda k, h=h0w: h[:, :, k, :])
                    per_t = -(-len(thunks) // S) if thunks else 0

                    if l1:
                        ps1 = ps1_tiles.pop(wi - 2)
                        hc1_prev = hc1
                        hc1 = h1pool.tile([128, S, KH, BC], F16, name="hc1")
                    if l0:
                        ps0 = ps0_tiles.pop(wi)
                        hc0_new = h0pool.tile([128, S, KH, BC], F16,
                                              name="hc0")
                    ti = 0
                    for t in range(S):
                        if l1:
                            if t == 0:
                                h1p = hz if wi == 2 else hc1_prev[:, S - 1]
                            else:
                                h1p = hc1[:, t - 1]
                            step(ps1, whh1, h1p, hc1, t)
                        if l0:
                            if t == 0:
                                h0p = hz if wi == 0 else \
                                    hc0_tiles[wi - 1][:, S - 1]
                            else:
                                h0p = hc0_new[:, t - 1]
                            step(ps0, whh0, h0p, hc0_new, t)
                        while ti < min(len(thunks), (t + 1) * per_t):
                            thunks[ti]()
                            ti += 1
                    while ti < len(thunks):
                        thunks[ti]()
                        ti += 1
                    if l0:
                        hc0_tiles.pop(wi - 2, None)
                        hc0_tiles[wi] = hc0_new

                # ---- final FC ----
                h1f = hc1[:, S - 1]
                psf = psfpool.tile([128, MO * BC], F32, name="psft")
                for m in range(MO):
                    for k in range(KH):
                        nc.tensor.matmul(
                            psf[:, m * BC:(m + 1) * BC],
                            wsl(wfc, k, m, MO),
                            h1f[:, k, :],
                            start=(m == 0 and k == 0),
                            stop=(m == MO - 1 and k == KH - 1),
                        )
                outs = spool.tile([128, MO * BC], F32, name="outs")
                for m in range(MO):
                    nc.scalar.activation(
                        outs[:, m * BC:(m + 1) * BC],
                        psf[:, m * BC:(m + 1) * BC],
                        AF.Identity,
                        bias=bfc[:, m:m + 1],
                    )
                for m in range(MO):
                    nc.sync.dma_start(out_d[m * 128:(m + 1) * 128, :],
                                      outs[:, m * BC:(m + 1) * BC])

    nc.compile()
    return nc


def make_in_maps(inputs, T=T):
    x = np.asarray(inputs["x"], np.float32)

    def t16(a):
        return np.ascontiguousarray(np.asarray(a, np.float32).T
                                    .astype(np.float16))

    shared = {
        "wih0T": t16(inputs["W_ih0"]),
        "whh0T": t16(inputs["W_hh0"]),
        "wih1T": t16(inputs["W_ih1"]),
        "whh1T": t16(inputs["W_hh1"]),
        "wfcT": t16(inputs["W_fc"]),
        "b0": (np.asarray(inputs["b_ih0"], np.float32)
               + np.asarray(inputs["b_hh0"], np.float32))
        .astype(np.float16).reshape(1, H),
        "b1": (np.asarray(inputs["b_ih1"], np.float32)
               + np.asarray(inputs["b_hh1"], np.float32))
        .astype(np.float16).reshape(1, H),
        "bfc": np.ascontiguousarray(
            np.asarray(inputs["b_fc"], np.float32).reshape(MO, 128).T),
    }
    in_maps = []
    for i in range(N_CORES):
        xc = x[i * BC:(i + 1) * BC, :T]
        xT = np.ascontiguousarray(
            xc.transpose(2, 1, 0).reshape(D, T * BC).astype(np.float16))
        in_maps.append({"xT": xT, **shared})
    return in_maps


def assemble_out(results):
    out = np.empty((B, O), np.float32)
    for i in range(N_CORES):
        out[i * BC:(i + 1) * BC] = results[i]["out"].T
    return out


_NC_CACHE = {}


def kernel(**inputs) -> np.ndarray:
    if "nc" not in _NC_CACHE:
        _NC_CACHE["nc"] = build()
    nc = _NC_CACHE["nc"]
    in_maps = make_in_maps(inputs)
    res = bass_utils.run_bass_kernel_spmd(nc, in_maps, list(range(N_CORES)))
    return assemble_out(res.results)
